# revision 3
# baseline (speedup 1.0000x reference)
"""Causal self-attention on 8 Trainium2 NeuronCores (Bass/Tile).

Problem: x[4,2048,1024] @ W_attn[1024,3072] + b_attn -> qkv; 16-head causal
attention; y @ W_proj[1024,1024] + b_proj.

Sharding: 2D over (batch, head-group). Core c = (b = c//2, g = c%2); each
core computes q/k/v for its 8 heads over its batch, causal attention (no max
subtraction -- logits are small -- with the softmax denominator accumulated
as a 65th "ones" column of v), then a partial output projection with its
512-row slice of W_proj. Host adds the two partials per batch plus b_proj.

v2 layout: all matmuls in bf16; x arrives bf16 and is transposed by the DMA
xbar (no PE transposes); W_attn/W_proj slices are resident in SBUF; the
QKV projection, attention, and output projection are interleaved per
512-token chunk so the scalar engine's exp overlaps PE projection work.
"""

import numpy as np

import concourse.bass as bass
import concourse.mybir as mybir
import concourse.tile as tile
from concourse import bacc
from concourse.bass_utils import run_bass_kernel_spmd

F32 = mybir.dt.float32
F32R = mybir.dt.float32r
BF16 = mybir.dt.bfloat16

B, T, D, H = 4, 2048, 1024, 16
HD = D // H               # 64
N_GROUPS = 2
FQ = D // N_GROUPS        # 512 features (8 heads) per core
N_CORES = B * N_GROUPS

# set by test harness to collect a trace / HW exec time
TRACE = False
LAST_RESULTS = None


def build_nc2(T=T, D=D, FQ=FQ, HD=HD, reps=1, GW=2,
              est_bufs=3, psc_bufs=2, ps_bufs=2, psy_bufs=2, xt_bufs=2,
              out_bufs=3):
    P = 128
    DCH = D // P          # 8 contraction chunks
    TCH = 512             # token chunk
    NTC = T // TCH        # 4
    NFB = FQ // P         # 4 feature blocks (q/k)
    HLOC = FQ // HD       # 8 local heads
    HPB = P // HD         # 2 heads per feature block
    JPQ = TCH // P        # 4 key blocks per token chunk
    NLC = FQ // P         # 4 dloc chunks for W_proj
    DOUT_CH = 512
    NDOUT = D // DOUT_CH  # 2
    scale = 1.0 / float(np.sqrt(HD))

    nc = bacc.Bacc()
    xb = nc.dram_tensor("xb", [T, D], BF16, kind="ExternalInput")
    wq = nc.dram_tensor("wq", [D, FQ], BF16, kind="ExternalInput")
    wk = nc.dram_tensor("wk", [D, FQ], BF16, kind="ExternalInput")
    wv = nc.dram_tensor("wv", [D, FQ], BF16, kind="ExternalInput")
    bq = nc.dram_tensor("bq", [FQ], F32, kind="ExternalInput")
    bk = nc.dram_tensor("bk", [FQ], F32, kind="ExternalInput")
    bv = nc.dram_tensor("bv", [FQ], BF16, kind="ExternalInput")
    wp = nc.dram_tensor("wp", [FQ, D], BF16, kind="ExternalInput")
    out = nc.dram_tensor("out", [T, D], F32, kind="ExternalOutput")

    with tile.TileContext(nc) as tc:
        with (
            tc.tile_pool(name="const", bufs=1) as const,
            tc.tile_pool(name="big", bufs=1) as big,
            tc.tile_pool(name="xtp", bufs=xt_bufs) as xtp,
            tc.tile_pool(name="est", bufs=est_bufs) as est,
            tc.tile_pool(name="small", bufs=3) as small,
            tc.tile_pool(name="outp", bufs=out_bufs) as outp,
            tc.tile_pool(name="ps", bufs=ps_bufs, space="PSUM") as ps,
            tc.tile_pool(name="psc", bufs=psc_bufs, space="PSUM") as psc,
            tc.tile_pool(name="psy", bufs=psy_bufs, space="PSUM") as psy,
        ):
            ones_f32 = const.tile([1, P], F32, tag="ones_f32")
            nc.vector.memset(ones_f32, 1.0)
            ones_row = const.tile([1, P], BF16)
            nc.vector.tensor_copy(out=ones_row, in_=ones_f32)
            # diagonal-block masks: mask_r[p, f] = 1 if f >= p + P*r else 0
            masks = []
            for r in range(JPQ):
                m = const.tile([P, TCH], BF16, tag=f"mask{r}")
                nc.gpsimd.memset(m, 1.0)
                nc.gpsimd.affine_select(
                    out=m, in_=m,
                    compare_op=mybir.AluOpType.is_ge,
                    fill=0.0,
                    base=-P * r,
                    pattern=[[1, TCH]],
                    channel_multiplier=-1,
                )
                masks.append(m)
            bq_sb = const.tile([P, NFB], F32, tag="bq")
            nc.sync.dma_start(out=bq_sb, in_=bq.rearrange("(o p) -> p o", p=P))
            bk_sb = const.tile([P, NFB], F32, tag="bk")
            nc.sync.dma_start(out=bk_sb, in_=bk.rearrange("(o p) -> p o", p=P))
            bv_sb = const.tile([1, FQ], BF16)
            nc.sync.dma_start(out=bv_sb, in_=bv[None, :])

            # resident weights (feature d = dc*128 + p, matching the DMA
            # transpose's row mapping)
            wq_sb = big.tile([P, DCH, FQ], BF16, tag="wq_sb")
            nc.sync.dma_start(out=wq_sb, in_=wq.rearrange("(dc p) f -> p dc f", p=P))
            wk_sb = big.tile([P, DCH, FQ], BF16, tag="wk_sb")
            nc.sync.dma_start(out=wk_sb, in_=wk.rearrange("(dc p) f -> p dc f", p=P))
            wv_sb = big.tile([P, DCH, FQ], BF16, tag="wv_sb")
            nc.sync.dma_start(out=wv_sb, in_=wv.rearrange("(dc p) f -> p dc f", p=P))
            wp_sb = big.tile([P, NLC, D], BF16, tag="wp_sb")
            nc.sync.dma_start(out=wp_sb, in_=wp.rearrange("(lc p) o -> p lc o", p=P))

            qT = big.tile([P, NFB, T], BF16, tag="qT")       # [f%128, fb, tok]
            kT = big.tile([P, NFB, T], BF16, tag="kT")
            v_aug = big.tile([P, T // P, HLOC, HD + 1], BF16, tag="v")
            yT = big.tile([P, NLC, T], BF16, tag="yT")       # [dloc%128, lc, tok]

            nc.vector.memset(v_aug[:, :, :, HD:HD + 1], 1.0)

            for _rep in range(reps):
              for tch in range(NTC):
                t0 = tch * TCH
                # --- stage A: DMA-transpose x chunk, project q/k/v ---
                xT = xtp.tile([P, DCH, TCH], BF16, tag="xT")
                nc.sync.dma_start_transpose(out=xT, in_=xb[t0:t0 + TCH, :])
                for (w_sb, bias_sb, dstT) in (
                        (wq_sb, bq_sb, qT), (wk_sb, bk_sb, kT)):
                    for fb in range(NFB):
                        pq = ps.tile([P, 512], F32, tag="ps")
                        for dc in range(DCH):
                            nc.tensor.matmul(
                                pq[:, :TCH],
                                w_sb[:, dc, fb * P:(fb + 1) * P],
                                xT[:, dc, :],
                                start=(dc == 0), stop=(dc == DCH - 1),
                            )
                        nc.vector.tensor_scalar_add(
                            out=dstT[:, fb, t0:t0 + TCH], in0=pq[:, :TCH],
                            scalar1=bias_sb[:, fb:fb + 1],
                        )
                for tb in range(TCH // P):
                    pv = ps.tile([P, 512], F32, tag="ps")
                    for dc in range(DCH):
                        nc.tensor.matmul(
                            pv[:, :FQ],
                            xT[:, dc, tb * P:(tb + 1) * P],
                            wv_sb[:, dc, :],
                            start=(dc == 0), stop=False,
                        )
                    nc.tensor.matmul(
                        pv[:, :FQ], ones_row, bv_sb, start=False, stop=True)
                    nc.vector.tensor_copy(
                        out=v_aug[:, tch * JPQ + tb, :, 0:HD],
                        in_=pv[:, :FQ].rearrange("p (h d) -> p h d", d=HD),
                    )

                # --- stage C: causal attention for query chunk tch ---
                NJ = (tch + 1) * JPQ
                ngrp = NJ // GW
                for h in range(HLOC):
                    fb = h // HPB
                    p0 = (h % HPB) * HD
                    py = psy.tile([P, 512], F32, tag="psy")
                    for g in range(ngrp):
                        pg = psc.tile([P, GW * 512], F32, tag="psc")
                        for jj in range(GW):
                            j = g * GW + jj
                            nc.tensor.matmul(
                                pg[:, jj * 512:(jj + 1) * 512],
                                kT[p0:p0 + HD, fb, j * P:(j + 1) * P],
                                qT[p0:p0 + HD, fb, t0:t0 + TCH],
                                start=True, stop=True,
                            )
                        eb = est.tile([P, GW * 512], BF16, tag="est")
                        nc.scalar.activation(
                            out=eb, in_=pg,
                            func=mybir.ActivationFunctionType.Exp,
                            scale=scale,
                        )
                        for jj in range(GW):
                            r = g * GW + jj - tch * JPQ
                            if r >= 0:
                                nc.vector.tensor_mul(
                                    out=eb[:, jj * 512:(jj + 1) * 512],
                                    in0=eb[:, jj * 512:(jj + 1) * 512],
                                    in1=masks[r],
                                )
                        for jj in range(GW):
                            j = g * GW + jj
                            nc.tensor.matmul(
                                py[:HD + 1, :TCH],
                                v_aug[:, j, h, :],
                                eb[:, jj * 512:(jj + 1) * 512],
                                start=(j == 0), stop=(j == NJ - 1),
                            )
                    recip = small.tile([1, TCH], F32, tag="recip")
                    nc.vector.reciprocal(out=recip, in_=py[HD:HD + 1, :TCH])
                    bcast = small.tile([HD, TCH], F32, tag="bcast")
                    nc.gpsimd.partition_broadcast(bcast, recip)
                    nc.vector.tensor_mul(
                        out=yT[p0:p0 + HD, fb, t0:t0 + TCH],
                        in0=py[:HD, :TCH],
                        in1=bcast,
                    )

                # --- stage D: output projection for this chunk ---
                for tb in range(TCH // P):
                    tbg = tch * JPQ + tb
                    for o in range(NDOUT):
                        po = ps.tile([P, 512], F32, tag="ps")
                        for i in range(NLC):
                            nc.tensor.matmul(
                                po[:, :DOUT_CH],
                                yT[:, i, tbg * P:(tbg + 1) * P],
                                wp_sb[:, i, o * DOUT_CH:(o + 1) * DOUT_CH],
                                start=(i == 0), stop=(i == NLC - 1),
                            )
                        ot = outp.tile([P, DOUT_CH], F32, tag="out")
                        nc.vector.tensor_copy(out=ot, in_=po[:, :DOUT_CH])
                        nc.sync.dma_start(
                            out=out[tbg * P:(tbg + 1) * P,
                                    o * DOUT_CH:(o + 1) * DOUT_CH],
                            in_=ot,
                        )

    nc.finalize()
    return nc


def build_nc3(T=T, D=D, FQ=FQ, HD=HD, reps=1, GW=2,
              est_bufs=3, psc_bufs=2, ps_bufs=2, psy_bufs=2, xt_bufs=2,
              out_bufs=3):
    """v3: same math as v2 but software-pipelined.

    - weight DMAs split per feature block and ordered after the first x
      chunk's transpose, so the first QKV matmul starts ~3.6us in;
    - within an attention head, scores for group g+1 are emitted before
      the AV matmuls of group g (PE never waits on the exp round-trip);
    - projection units for chunk tch+1 and output-projection units for
      chunk tch-1 are interleaved between attention heads of chunk tch,
      giving PE independent work while Act drains exps.
    """
    P = 128
    DCH = D // P
    TCH = 512
    NTC = T // TCH
    NFB = FQ // P
    HLOC = FQ // HD
    HPB = P // HD
    JPQ = TCH // P
    NLC = FQ // P
    DOUT_CH = 512
    NDOUT = D // DOUT_CH
    scale = 1.0 / float(np.sqrt(HD))

    nc = bacc.Bacc()
    xb = nc.dram_tensor("xb", [T, D], BF16, kind="ExternalInput")
    wq = nc.dram_tensor("wq", [D, FQ], BF16, kind="ExternalInput")
    wk = nc.dram_tensor("wk", [D, FQ], BF16, kind="ExternalInput")
    wv = nc.dram_tensor("wv", [D, FQ], BF16, kind="ExternalInput")
    bq = nc.dram_tensor("bq", [FQ], F32, kind="ExternalInput")
    bk = nc.dram_tensor("bk", [FQ], F32, kind="ExternalInput")
    bv = nc.dram_tensor("bv", [FQ], BF16, kind="ExternalInput")
    wp = nc.dram_tensor("wp", [FQ, D], BF16, kind="ExternalInput")
    out = nc.dram_tensor("out", [T, D], F32, kind="ExternalOutput")

    with tile.TileContext(nc) as tc:
        with (
            tc.tile_pool(name="const", bufs=1) as const,
            tc.tile_pool(name="big", bufs=1) as big,
            tc.tile_pool(name="xtp", bufs=xt_bufs) as xtp,
            tc.tile_pool(name="est", bufs=est_bufs) as est,
            tc.tile_pool(name="small", bufs=3) as small,
            tc.tile_pool(name="outp", bufs=out_bufs) as outp,
            tc.tile_pool(name="ps", bufs=ps_bufs, space="PSUM") as ps,
            tc.tile_pool(name="psc", bufs=psc_bufs, space="PSUM") as psc,
            tc.tile_pool(name="psy", bufs=psy_bufs, space="PSUM") as psy,
        ):
            # --- tiles ---
            xts = [xtp.tile([P, DCH, TCH], BF16, tag=f"xT{i % xt_bufs}")
                   for i in range(NTC)] if False else None
            wq_sb = big.tile([P, DCH, FQ], BF16, tag="wq_sb")
            wk_sb = big.tile([P, DCH, FQ], BF16, tag="wk_sb")
            wv_sb = big.tile([P, DCH, FQ], BF16, tag="wv_sb")
            wp_sb = big.tile([P, NLC, D], BF16, tag="wp_sb")
            qT = big.tile([P, NFB, T], BF16, tag="qT")
            kT = big.tile([P, NFB, T], BF16, tag="kT")
            v_aug = big.tile([P, T // P, HLOC, HD + 1], BF16, tag="v")
            yT = big.tile([P, NLC, T], BF16, tag="yT")

            # --- first x chunk transpose goes out first, then weights ---
            xT0 = xtp.tile([P, DCH, TCH], BF16, tag="xT")
            nc.sync.dma_start_transpose(out=xT0, in_=xb[0:TCH, :])
            # wq/wk split per feature block so fb=0 matmuls start early
            for fb in range(NFB):
                nc.sync.dma_start(
                    out=wq_sb[:, :, fb * P:(fb + 1) * P],
                    in_=wq.rearrange("(dc p) f -> p dc f", p=P)[
                        :, :, fb * P:(fb + 1) * P])
            bq_sb = const.tile([P, NFB], F32, tag="bq")
            nc.sync.dma_start(out=bq_sb, in_=bq.rearrange("(o p) -> p o", p=P))
            for fb in range(NFB):
                nc.sync.dma_start(
                    out=wk_sb[:, :, fb * P:(fb + 1) * P],
                    in_=wk.rearrange("(dc p) f -> p dc f", p=P)[
                        :, :, fb * P:(fb + 1) * P])
            bk_sb = const.tile([P, NFB], F32, tag="bk")
            nc.sync.dma_start(out=bk_sb, in_=bk.rearrange("(o p) -> p o", p=P))
            nc.sync.dma_start(out=wv_sb, in_=wv.rearrange("(dc p) f -> p dc f", p=P))
            bv_sb = const.tile([1, FQ], BF16)
            nc.sync.dma_start(out=bv_sb, in_=bv[None, :])
            nc.sync.dma_start(out=wp_sb, in_=wp.rearrange("(lc p) o -> p lc o", p=P))

            ones_f32 = const.tile([1, P], F32, tag="ones_f32")
            nc.vector.memset(ones_f32, 1.0)
            ones_row = const.tile([1, P], BF16)
            nc.vector.tensor_copy(out=ones_row, in_=ones_f32)
            masks = []
            for r in range(JPQ):
                m = const.tile([P, TCH], BF16, tag=f"mask{r}")
                nc.gpsimd.memset(m, 1.0)
                nc.gpsimd.affine_select(
                    out=m, in_=m,
                    compare_op=mybir.AluOpType.is_ge,
                    fill=0.0,
                    base=-P * r,
                    pattern=[[1, TCH]],
                    channel_multiplier=-1,
                )
                masks.append(m)
            nc.vector.memset(v_aug[:, :, :, HD:HD + 1], 1.0)

            for _rep in range(reps):
              xT_cur = {0: xT0 if _rep == 0 else None}

              def emit_xt(tch):
                  t0 = tch * TCH
                  xT = xtp.tile([P, DCH, TCH], BF16, tag="xT")
                  nc.sync.dma_start_transpose(out=xT, in_=xb[t0:t0 + TCH, :])
                  xT_cur[tch] = xT

              if xT_cur[0] is None:
                  emit_xt(0)

              def a_units(tch):
                  """QKV projection units for chunk tch (12 closures)."""
                  t0 = tch * TCH
                  units = []

                  def qk_unit(w_sb, bias_sb, dstT, fb):
                      def run():
                          xT = xT_cur[tch]
                          pq = ps.tile([P, 512], F32, tag="ps")
                          for dc in range(DCH):
                              nc.tensor.matmul(
                                  pq[:, :TCH],
                                  w_sb[:, dc, fb * P:(fb + 1) * P],
                                  xT[:, dc, :],
                                  start=(dc == 0), stop=(dc == DCH - 1),
                              )
                          nc.vector.tensor_scalar_add(
                              out=dstT[:, fb, t0:t0 + TCH], in0=pq[:, :TCH],
                              scalar1=bias_sb[:, fb:fb + 1],
                          )
                      return run

                  def v_unit(tb):
                      def run():
                          xT = xT_cur[tch]
                          pv = ps.tile([P, 512], F32, tag="ps")
                          for dc in range(DCH):
                              nc.tensor.matmul(
                                  pv[:, :FQ],
                                  xT[:, dc, tb * P:(tb + 1) * P],
                                  wv_sb[:, dc, :],
                                  start=(dc == 0), stop=False,
                              )
                          nc.tensor.matmul(
                              pv[:, :FQ], ones_row, bv_sb,
                              start=False, stop=True)
                          nc.vector.tensor_copy(
                              out=v_aug[:, tch * JPQ + tb, :, 0:HD],
                              in_=pv[:, :FQ].rearrange("p (h d) -> p h d", d=HD),
                          )
                      return run

                  # k first: attention h=0 needs kT/qT fb=0 earliest
                  for fb in range(NFB):
                      units.append(qk_unit(wk_sb, bk_sb, kT, fb))
                      units.append(qk_unit(wq_sb, bq_sb, qT, fb))
                  for tb in range(TCH // P):
                      units.append(v_unit(tb))
                  return units

              def c_unit(tch, h):
                  """One attention head for query chunk tch."""
                  t0 = tch * TCH
                  NJ = (tch + 1) * JPQ
                  ngrp = NJ // GW

                  def run():
                      fb = h // HPB
                      p0 = (h % HPB) * HD
                      py = psy.tile([P, 512], F32, tag="psy")
                      ebs = {}

                      def scores(g):
                          pg = psc.tile([P, GW * 512], F32, tag="psc")
                          for jj in range(GW):
                              j = g * GW + jj
                              nc.tensor.matmul(
                                  pg[:, jj * 512:(jj + 1) * 512],
                                  kT[p0:p0 + HD, fb, j * P:(j + 1) * P],
                                  qT[p0:p0 + HD, fb, t0:t0 + TCH],
                                  start=True, stop=True,
                              )
                          eb = est.tile([P, GW * 512], BF16, tag="est")
                          nc.scalar.activation(
                              out=eb, in_=pg,
                              func=mybir.ActivationFunctionType.Exp,
                              scale=scale,
                          )
                          for jj in range(GW):
                              r = g * GW + jj - tch * JPQ
                              if r >= 0:
                                  nc.vector.tensor_mul(
                                      out=eb[:, jj * 512:(jj + 1) * 512],
                                      in0=eb[:, jj * 512:(jj + 1) * 512],
                                      in1=masks[r],
                                  )
                          ebs[g] = eb

                      def av(g):
                          eb = ebs.pop(g)
                          for jj in range(GW):
                              j = g * GW + jj
                              nc.tensor.matmul(
                                  py[:HD + 1, :TCH],
                                  v_aug[:, j, h, :],
                                  eb[:, jj * 512:(jj + 1) * 512],
                                  start=(j == 0), stop=(j == NJ - 1),
                              )

                      scores(0)
                      for g in range(1, ngrp):
                          scores(g)
                          av(g - 1)
                      av(ngrp - 1)

                      recip = small.tile([1, TCH], F32, tag="recip")
                      nc.vector.reciprocal(out=recip, in_=py[HD:HD + 1, :TCH])
                      bcast = small.tile([HD, TCH], F32, tag="bcast")
                      nc.gpsimd.partition_broadcast(bcast, recip)
                      nc.vector.tensor_mul(
                          out=yT[p0:p0 + HD, fb, t0:t0 + TCH],
                          in0=py[:HD, :TCH],
                          in1=bcast,
                      )
                  return run

              def d_units(tch):
                  units = []

                  def d_unit(tb, o):
                      def run():
                          tbg = tch * JPQ + tb
                          po = ps.tile([P, 512], F32, tag="ps")
                          for i in range(NLC):
                              nc.tensor.matmul(
                                  po[:, :DOUT_CH],
                                  yT[:, i, tbg * P:(tbg + 1) * P],
                                  wp_sb[:, i, o * DOUT_CH:(o + 1) * DOUT_CH],
                                  start=(i == 0), stop=(i == NLC - 1),
                              )
                          ot = outp.tile([P, DOUT_CH], F32, tag="out")
                          nc.vector.tensor_copy(out=ot, in_=po[:, :DOUT_CH])
                          nc.sync.dma_start(
                              out=out[tbg * P:(tbg + 1) * P,
                                      o * DOUT_CH:(o + 1) * DOUT_CH],
                              in_=ot,
                          )
                      return run

                  for tb in range(TCH // P):
                      for o in range(NDOUT):
                          units.append(d_unit(tb, o))
                  return units

              # schedule: A(0); then per tch, attention heads with A(tch+1)
              # and D(tch-1) units as filler; D(last) at the end.
              for u in a_units(0):
                  u()
              for tch in range(NTC):
                  filler = []
                  if tch + 1 < NTC:
                      emit_xt(tch + 1)
                      filler += a_units(tch + 1)
                  if tch >= 1:
                      filler += d_units(tch - 1)
                  nfill = len(filler)
                  fi = 0
                  for h in range(HLOC):
                      c_unit(tch, h)()
                      # spread filler over the 8 heads
                      want = (h + 1) * nfill // HLOC
                      while fi < want:
                          filler[fi]()
                          fi += 1
                  while fi < nfill:
                      filler[fi]()
                      fi += 1
              for u in d_units(NTC - 1):
                  u()

    nc.finalize()
    return nc


DEFAULT_CFG = dict()

_NC_CACHE = {}


def _get_nc():
    if "nc" not in _NC_CACHE:
        _NC_CACHE["nc"] = build_nc3(**DEFAULT_CFG)
    return _NC_CACHE["nc"]


def _core_inputs(inputs):
    import ml_dtypes
    bf16 = ml_dtypes.bfloat16
    x = np.asarray(inputs["x"], dtype=np.float32)
    W = np.asarray(inputs["W_attn"], dtype=np.float32)
    ba = np.asarray(inputs["b_attn"], dtype=np.float32)
    Wp = np.asarray(inputs["W_proj"], dtype=np.float32)
    maps = []
    for c in range(N_CORES):
        b, g = c // N_GROUPS, c % N_GROUPS
        s = slice(g * FQ, (g + 1) * FQ)
        maps.append({
            "xb": np.ascontiguousarray(x[b]).astype(bf16),
            "wq": np.ascontiguousarray(W[:, 0:D][:, s]).astype(bf16),
            "wk": np.ascontiguousarray(W[:, D:2 * D][:, s]).astype(bf16),
            "wv": np.ascontiguousarray(W[:, 2 * D:3 * D][:, s]).astype(bf16),
            "bq": np.ascontiguousarray(ba[0:D][s]),
            "bk": np.ascontiguousarray(ba[D:2 * D][s]),
            "bv": np.ascontiguousarray(ba[2 * D:3 * D][s]).astype(bf16),
            "wp": np.ascontiguousarray(Wp[s, :]).astype(bf16),
        })
    return maps


def kernel(**inputs) -> np.ndarray:
    global LAST_RESULTS
    nc = _get_nc()
    maps = _core_inputs(inputs)
    res = run_bass_kernel_spmd(
        nc, maps, list(range(N_CORES)), trace=TRACE,
        trace_cores=list(range(N_CORES)) if TRACE else None,
    )
    LAST_RESULTS = res
    bp = np.asarray(inputs["b_proj"], dtype=np.float32)
    out = np.empty((B, T, D), dtype=np.float32)
    for b in range(B):
        acc = res.results[b * N_GROUPS]["out"].astype(np.float32).copy()
        for g in range(1, N_GROUPS):
            acc += res.results[b * N_GROUPS + g]["out"]
        out[b] = acc + bp
    return out
